# revision 1
# baseline (speedup 1.0000x reference)
"""CVAE (3-layer LSTM enc + 3-layer LSTM dec) Trainium2 kernel.

Strategy: 8-way data-parallel over batch (B=256 -> 32/core). Per core a raw
Bass program runs both LSTM stacks fully unrolled over T=128 steps.

Layout/maths tricks:
  * Embedding + layer-0 input matmul folded on host into a 42-row table
    (emb @ k0) gathered per (b,t) -> per-step "xc" tensors streamed from DRAM.
  * Per step, gate pre-activations z[32, 2048] are computed as
    lhsT.T @ rhs with lhsT = transposed-state tiles hT [128k, 32] (bf16) and
    rhs = naturally-laid-out weights [128k, 2048] (bf16), PSUM fp32.
  * States h are transposed each step with PE transpose-mode matmuls
    (identity rhs), cast to bf16 during the PSUM->SBUF copy.
  * Losses reduced on device to per-core partial sums; final scalar combine
    on host (no collectives).
"""

import sys as _sys

for _p in ("/opt/trn_rl_repo",):
    if _p not in _sys.path:
        _sys.path.insert(0, _p)

from contextlib import ExitStack

import numpy as np
import ml_dtypes

import concourse.bass as bass
import concourse.mybir as mybir
from concourse.bass_utils import run_bass_kernel_spmd

bf16 = ml_dtypes.bfloat16
F32 = mybir.dt.float32
BF = mybir.dt.bfloat16

B, T, V, LAT, U, P = 256, 128, 42, 200, 512, 3
NCORES = 8
BL = B // NCORES            # 32 local batch
G4 = 4 * U                  # 2048 gate width
KT = U // 128               # 4 K-tiles per 512
NBANK = 4                   # 512-wide N chunks of z
AF = mybir.ActivationFunctionType
OP = mybir.AluOpType

XC_BUFS = 4
DEBUG_DUMP = False


class Tracker:
    """Records semaphore increment values by tag (pass 1), serves them (pass 2)."""

    def __init__(self):
        self.vals = {}
        self.counts = {}

    def inc(self, sem, tag, n=1):
        c = self.counts.get(sem, 0) + n
        self.counts[sem] = c
        key = (sem, tag)
        assert key not in self.vals, f"dup inc tag {key}"
        self.vals[key] = c
        return c

    def val(self, sem, tag):
        return self.vals[(sem, tag)]


def _build(flags):
    nc = bass.Bass()

    # ---- DRAM params ----
    dp = lambda n, s, d, o=False: nc.declare_dram_parameter(n, list(s), d, isOutput=o)
    xc_enc_d = dp("xc_enc", [T, BL, G4], BF)
    xc_dec_d = dp("xc_dec", [T, BL, G4], BF)
    rk0e_d = dp("rk0e", [128, KT, G4], BF)
    w1e_d = dp("w1e", [128, 2 * KT, G4], BF)
    w2e_d = dp("w2e", [128, 2 * KT, G4], BF)
    rk0d_d = dp("rk0d", [128, KT, G4], BF)
    w1d_d = dp("w1d", [128, 2 * KT, G4], BF)
    w2d_d = dp("w2d", [128, 2 * KT, G4], BF)
    dk0z_d = dp("dk0z", [128, 2, G4], BF)
    wm_d = dp("wm", [128, KT, LAT], BF)
    ws_d = dp("ws", [128, KT, LAT], BF)
    wo_d = dp("wo", [128, KT, V], BF)
    ohm_d = dp("ohm", [BL, T * V], BF)
    maskf_d = dp("maskf", [BL, T], F32)
    eps_d = dp("eps", [BL, LAT], F32)
    iden_d = dp("iden", [BL, BL], F32)
    ones32_d = dp("ones32", [BL, 1], F32)
    out_d = dp("out", [1, 4], F32, o=True)
    dbg_d = dp("dbg", [BL, 2 * T], F32, o=True) if DEBUG_DUMP else None
    bias_d = {}
    for nm in ("b1e", "b2e", "b1d", "b2d"):
        if flags[nm]:
            bias_d[nm] = dp(nm, [1, G4], BF)
    for nm, w in (("bm", LAT), ("bs", LAT), ("bo", V)):
        if flags[nm]:
            bias_d[nm] = dp(nm, [1, w], BF)
    if any(flags[n] for n in flags):
        bias_d["ones1"] = dp("ones1", [1, BL], BF)

    tk = Tracker()

    with ExitStack() as ctx:
        _nn = [0]

        def sbt(*s):
            _nn[0] += 1
            return ctx.enter_context(
                nc.sbuf_tensor(f"sb{_nn[0]}", list(s[:-1]), s[-1]))

        def pst(*s):
            _nn[0] += 1
            return ctx.enter_context(
                nc.psum_tensor(f"ps{_nn[0]}", list(s[:-1]), s[-1]))

        # ---- SBUF ----
        rk0_sb = sbt(128, KT * G4, BF)          # enc then dec
        w1_sb = sbt(128, 2 * KT * G4, BF)
        w2_sb = sbt(128, 2 * KT * G4, BF)
        dk0z_sb = sbt(128, 2 * G4, BF)
        wm_sb = sbt(128, KT * LAT, BF)
        ws_sb = sbt(128, KT * LAT, BF)
        wo_sb = sbt(128, KT * V, BF)
        xc_sb = [sbt(BL, G4, BF) for _ in range(XC_BUFS)]
        hT = [sbt(128, KT * BL, BF) for _ in range(3)]   # per live layer
        cst = [sbt(BL, U, F32) for _ in range(3)]
        h2_sb = [sbt(BL, U, F32) for _ in range(2)]      # parity ping-pong
        sig_if = [sbt(BL, 2 * U, F32) for _ in range(2)]
        tanh_g = [sbt(BL, U, F32) for _ in range(2)]
        sig_o = [sbt(BL, U, F32) for _ in range(2)]
        tanhc = [sbt(BL, U, F32) for _ in range(2)]
        fc_sb = sbt(BL, U, F32)
        ig_sb = sbt(BL, U, F32)
        zpart_sb = sbt(BL, G4, F32)
        iden_sb = sbt(BL, BL, F32)
        ones32_sb = sbt(BL, 1, F32)
        ohm_sb = sbt(BL, T * V, BF)
        maskf_sb = sbt(BL, T, F32)
        eps_sb = sbt(BL, LAT, F32)
        mean_sb = sbt(BL, LAT, F32)
        ls_sb = sbt(BL, LAT, F32)
        els2_sb = sbt(BL, LAT, F32)
        zr_sb = sbt(BL, LAT, F32)
        cT_sb = sbt(128, KT * BL, BF)
        zT_sb = sbt(128, 2 * BL, BF)
        pre_buf = sbt(BL, T * V, F32)
        se_sb = sbt(BL, T, F32)
        m2_sb = sbt(BL, LAT, F32)
        els_sb = sbt(BL, LAT, F32)
        partial_sb = sbt(BL, 4, F32)
        out_sb = sbt(1, 4, F32)
        bias_sb = {}
        for nm, t in bias_d.items():
            bias_sb[nm] = sbt(1, t.shape[-1], BF)

        # ---- PSUM (allocation order == bank order) ----
        z_ps = pst(BL, G4, F32)        # banks 0-3
        trP = [pst(128, 512, F32), pst(128, 512, F32)]  # banks 4,5
        preP = [pst(BL, 512, F32), pst(BL, 512, F32)]   # banks 6,7

        s_pe = ctx.enter_context(nc.semaphore("s_pe"))
        s_act = ctx.enter_context(nc.semaphore("s_act"))
        s_dve = ctx.enter_context(nc.semaphore("s_dve"))
        s_xcb = [ctx.enter_context(nc.semaphore(f"s_xc{i}"))
                 for i in range(XC_BUFS)]
        s_w = ctx.enter_context(nc.semaphore("s_w"))
        s_out = ctx.enter_context(nc.semaphore("s_out"))
        blk = ctx.enter_context(nc.Block())

        SEMS = {"pe": s_pe, "act": s_act, "dve": s_dve, "w": s_w,
                "out": s_out}
        for i in range(XC_BUFS):
            SEMS[f"xc{i}"] = s_xcb[i]

        # helpers -----------------------------------------------------------
        class Em:
            def __init__(self, real, eng=None):
                self.real = real
                self.eng = eng

            def wait(self, sem, tag):
                if self.real:
                    self.eng.wait_ge(SEMS[sem], tk.val(sem, tag))

            def wait_val(self, sem, v):
                if self.real:
                    self.eng.wait_ge(SEMS[sem], v)

            def inc(self, ins, sem, tag, n=1):
                v = None
                if not self.real:
                    v = tk.inc(sem, tag, n)
                else:
                    ins.then_inc(SEMS[sem], n)
                return v

        # global transpose-group counter is a function of phase:
        # enc: gg = 3*t + l  (0..383); p2 cT: 384, p2 zT: 385;
        # dec: gg = 386 + 3*t + l
        def gg_enc(t, l):
            return 3 * t + l

        GG_CT = 3 * T          # phase-2 c2 transpose group
        GG_ZT = 3 * T + 1      # phase-2 zr transpose group
        DEC_BASE = 3 * T + 2

        def gg_dec(t, l):
            return DEC_BASE + 3 * t + l

        def prev_gate_g(g):
            """Previous gate-group of same parity (GG_CT/GG_ZT are transpose-only)."""
            if g == DEC_BASE:
                return DEC_BASE - 4
            if g == DEC_BASE + 1:
                return DEC_BASE - 3
            return g - 2

        # which (sem, tag) marks "copies of transpose-group gg done"
        def copies_tag(gg):
            return ("copies", gg)

        def w_rhs(wsb, k, n):
            return wsb[:, k * G4 + n * 512:k * G4 + (n + 1) * 512]

        # ---- PE program ----------------------------------------------------
        def pe_layer_mms(em, ph, t, l, wsb, nk, stat_fn, bias_nm):
            """z[:, n] = sum_k stat_k.T @ W[k, n]  (+ bias row)."""
            for n in range(NBANK):
                # bank-free wait: previous layer-step's readers of bank n
                pg = (ph, t, l)
                prev = prev_zuser(ph, t, l)
                if prev is not None:
                    pph, pt, pl = prev
                    if n <= 2:
                        em.wait("act", ("ifg", pph, pt, pl))
                    else:
                        em.wait("act", ("o", pph, pt, pl))
                elif ph == "dec":
                    em.wait("act", ("p2_zpart_copy",))
                # stationary readiness (once, at bank 0)
                if n == 0:
                    st = stat_wait(ph, t, l)
                    for sem, tag in st:
                        em.wait(sem, tag)
                nb = flags[bias_nm] if bias_nm else False
                for k in range(nk):
                    last = (k == nk - 1) and not nb
                    if em.real:
                        ins = nc.tensor.matmul(
                            z_ps[:, n * 512:(n + 1) * 512],
                            stat_fn(k),
                            w_rhs(wsb, k, n),
                            start=(k == 0), stop=(k == nk - 1) and not nb)
                        if last:
                            ins.then_inc(s_pe, 1)
                    if last:
                        tk.inc("pe", ("bank", ph, t, l, n)) if not em.real else None
                if nb:
                    if em.real:
                        ins = nc.tensor.matmul(
                            z_ps[:, n * 512:(n + 1) * 512],
                            bias_sb["ones1"][:, :BL],
                            bias_sb[bias_nm][:, n * 512:(n + 1) * 512],
                            start=False, stop=True)
                        ins.then_inc(s_pe, 1)
                    else:
                        tk.inc("pe", ("bank", ph, t, l, n))

        def prev_zuser(ph, t, l):
            """(ph,t,l) of previous user of the z psum banks, or None."""
            if ph == "enc":
                if t == 0 and l == 0:
                    return None
                if l == 0:
                    return ("enc", t - 1, 2)
                return ("enc", t, l - 1)
            else:
                if t == 0 and l == 0:
                    return None  # guarded by p2_zpart_copy
                if l == 0:
                    return ("dec", t - 1, 2)
                return ("dec", t, l - 1)

        def stat_wait(ph, t, l):
            """sem/tags that make the stationary hT tiles valid."""
            out = []
            if ph == "enc":
                if l == 0:
                    if t == 0:
                        out.append(("dve", ("init",)))
                    else:
                        out.append(("dve", copies_tag(gg_enc(t - 1, 0))))
                else:
                    out.append(("dve", copies_tag(gg_enc(t, l - 1))))
            else:
                if l == 0:
                    if t == 0:
                        out.append(("dve", ("init2",)))
                    else:
                        out.append(("dve", copies_tag(gg_dec(t - 1, 0))))
                else:
                    out.append(("dve", copies_tag(gg_dec(t, l - 1))))
            return out

        def pe_transposes(em, gg, src_sb, ngrp=KT, tag=None):
            """Transpose src [BL, 128*ngrp] into trP[gg&1][:, j*32...]."""
            tp = trP[gg & 1]
            if gg >= 2:
                em.wait("dve", copies_tag(gg - 2))
            for j in range(ngrp):
                last = (j == ngrp - 1)
                if em.real:
                    ins = nc.tensor.transpose(
                        tp[:, j * BL:(j + 1) * BL],
                        src_sb[:, j * 128:(j + 1) * 128],
                        iden_sb[:])
                    if last:
                        ins.then_inc(s_pe, 1)
                elif last:
                    tk.inc("pe", ("tr", gg))

        def pe_body(e, real):
            em = Em(real, e)
            # wait weights: smalls + rk0e
            em.wait("w", ("w_rk0e",))
            for t in range(T):
                for l in range(3):
                    g = gg_enc(t, l)
                    if l == 0:
                        pe_layer_mms(em, "enc", t, 0, rk0_sb, KT,
                                     lambda k: hT[0][:, k * BL:(k + 1) * BL], None)
                    else:
                        if t == 0 and l == 1:
                            em.wait("w", ("w_w1e",))
                        if t == 0 and l == 2:
                            em.wait("w", ("w_w2e",))
                        wsb = w1_sb if l == 1 else w2_sb
                        bnm = "b1e" if l == 1 else "b2e"

                        def stat(k, l=l):
                            if k < KT:
                                return hT[l - 1][:, k * BL:(k + 1) * BL]
                            return hT[l][:, (k - KT) * BL:(k - KT + 1) * BL]
                        pe_layer_mms(em, "enc", t, l, wsb, 2 * KT, stat, bnm)
                    # transposes of this layer's h2
                    em.wait("dve", ("h2", "enc", t, l))
                    pe_transposes(em, g, h2_sb[g & 1])

            # ---- phase 2: mean/ls/reparam/zpart ----
            em.wait("dve", ("c2", "enc", T - 1, 2))
            pe_transposes(em, GG_CT, cst[2])
            em.wait("dve", ("p2_cT",))
            for dst, wsb, bn, tg in ((preP[0], wm_sb, "bm", "p2_mm_mean"),
                                     (preP[1], ws_sb, "bs", "p2_mm_ls")):
                nb = flags[bn]
                for k in range(KT):
                    last = (k == KT - 1) and not nb
                    if em.real:
                        ins = nc.tensor.matmul(
                            dst[:, 0:LAT], cT_sb[:, k * BL:(k + 1) * BL],
                            wsb[:, k * LAT:(k + 1) * LAT],
                            start=(k == 0), stop=last)
                        if last:
                            ins.then_inc(s_pe, 1)
                    elif last:
                        tk.inc("pe", (tg,))
                if nb:
                    if em.real:
                        nc.tensor.matmul(dst[:, 0:LAT], bias_sb["ones1"][:, :BL],
                                         bias_sb[bn][:], start=False,
                                         stop=True).then_inc(s_pe, 1)
                    else:
                        tk.inc("pe", (tg,))
            # zr transposes (2 groups: [32,128],[32,72])
            em.wait("dve", ("p2_zr",))
            em.wait("dve", copies_tag(GG_ZT - 2))
            tp = trP[GG_ZT & 1]
            if em.real:
                nc.tensor.transpose(tp[:, 0:BL], zr_sb[:, 0:128], iden_sb[:])
                ins = nc.tensor.transpose(tp[0:LAT - 128, BL:2 * BL],
                                          zr_sb[:, 128:LAT], iden_sb[:])
                ins.then_inc(s_pe, 1)
            else:
                tk.inc("pe", ("tr", GG_ZT))
            # zpart matmuls into z banks
            em.wait("dve", ("p2_zT",))
            em.wait("act", ("o", "enc", T - 1, 2))
            for n in range(NBANK):
                if em.real:
                    nc.tensor.matmul(z_ps[:, n * 512:(n + 1) * 512],
                                     zT_sb[:, 0:BL],
                                     dk0z_sb[:, n * 512:(n + 1) * 512],
                                     start=True, stop=False)
                    ins = nc.tensor.matmul(
                        z_ps[:, n * 512:(n + 1) * 512],
                        zT_sb[0:LAT - 128, BL:2 * BL],
                        dk0z_sb[0:LAT - 128, G4 + n * 512:G4 + (n + 1) * 512],
                        start=False, stop=True)
                    ins.then_inc(s_pe, 1)
                else:
                    tk.inc("pe", ("p2_zpart", n))

            # ---- dec loop ----
            em.wait("w", ("w_dec",))
            for t in range(T):
                for l in range(3):
                    g = gg_dec(t, l)
                    if l == 0:
                        pe_layer_mms(em, "dec", t, 0, rk0_sb, KT,
                                     lambda k: hT[0][:, k * BL:(k + 1) * BL], None)
                    else:
                        wsb = w1_sb if l == 1 else w2_sb
                        bnm = "b1d" if l == 1 else "b2d"

                        def stat(k, l=l):
                            if k < KT:
                                return hT[l - 1][:, k * BL:(k + 1) * BL]
                            return hT[l][:, (k - KT) * BL:(k - KT + 1) * BL]
                        pe_layer_mms(em, "dec", t, l, wsb, 2 * KT, stat, bnm)
                    em.wait("dve", ("h2", "dec", t, l))
                    pe_transposes(em, g, h2_sb[g & 1])
                # projection for step t
                em.wait("dve", copies_tag(gg_dec(t, 2)))
                if t >= 2:
                    em.wait("act", ("precopy", t - 2))
                elif t == 0:
                    em.wait("act", ("p2_mean_sb",))
                elif t == 1:
                    em.wait("act", ("p2_ls_sb",))
                pp = preP[t & 1]
                nb = flags["bo"]
                for k in range(KT):
                    last = (k == KT - 1) and not nb
                    if em.real:
                        ins = nc.tensor.matmul(
                            pp[:, 0:V], hT[2][:, k * BL:(k + 1) * BL],
                            wo_sb[:, k * V:(k + 1) * V],
                            start=(k == 0), stop=last)
                        if last:
                            ins.then_inc(s_pe, 1)
                    elif last:
                        tk.inc("pe", ("proj", t))
                if nb:
                    if em.real:
                        nc.tensor.matmul(pp[:, 0:V], bias_sb["ones1"][:, :BL],
                                         bias_sb["bo"][:], start=False,
                                         stop=True).then_inc(s_pe, 1)
                    else:
                        tk.inc("pe", ("proj", t))

            # ---- final partial reduce ----
            em.wait("dve", ("f_kl",))
            if em.real:
                nc.tensor.matmul(preP[0][0:1, 300:304], ones32_sb[:],
                                 partial_sb[:], start=True,
                                 stop=True).then_inc(s_pe, 1)
            else:
                tk.inc("pe", ("f_red",))

        # ---- ACT program ---------------------------------------------------
        def act_layer(em, ph, t, l):
            g = gg_enc(t, l) if ph == "enc" else gg_dec(t, l)
            par = g & 1
            if l == 0:
                em.wait("dve", ("xcadd", ph, t, 1))
            else:
                em.wait("pe", ("bank", ph, t, l, 1))
            if g >= 2:
                em.wait("dve", ("c2g", prev_gate_g(g)))
            if em.real:
                nc.scalar.activation(sig_if[par][:], z_ps[:, 0:2 * U], AF.Sigmoid)
            if l == 0:
                em.wait("dve", ("xcadd", ph, t, 2))
            else:
                em.wait("pe", ("bank", ph, t, l, 2))
            if em.real:
                ins = nc.scalar.activation(tanh_g[par][:], z_ps[:, 2 * U:3 * U],
                                           AF.Tanh)
                ins.then_inc(s_act, 1)
            else:
                tk.inc("act", ("ifg", ph, t, l))
            if l == 0:
                em.wait("dve", ("xcadd", ph, t, 3))
            else:
                em.wait("pe", ("bank", ph, t, l, 3))
            if g >= 2:
                em.wait("dve", ("h2g", prev_gate_g(g)))
            if em.real:
                ins = nc.scalar.activation(sig_o[par][:], z_ps[:, 3 * U:4 * U],
                                           AF.Sigmoid)
                ins.then_inc(s_act, 1)
            else:
                tk.inc("act", ("o", ph, t, l))
            em.wait("dve", ("c2", ph, t, l))
            if em.real:
                ins = nc.scalar.activation(tanhc[par][:], cst[l][:], AF.Tanh)
                ins.then_inc(s_act, 1)
            else:
                tk.inc("act", ("oc", ph, t, l))

        def act_body(e, real):
            em = Em(real, e)
            for t in range(T):
                for l in range(3):
                    act_layer(em, "enc", t, l)
            # phase 2
            em.wait("pe", ("p2_mm_mean",))
            if em.real:
                ins = nc.scalar.copy(mean_sb[:], preP[0][:, 0:LAT])
                ins.then_inc(s_act, 1)
            else:
                tk.inc("act", ("p2_mean_sb",))
            em.wait("pe", ("p2_mm_ls",))
            if em.real:
                ins = nc.scalar.copy(ls_sb[:], preP[1][:, 0:LAT])
                ins.then_inc(s_act, 1)
            else:
                tk.inc("act", ("p2_ls_sb",))
            if em.real:
                ins = nc.scalar.activation(els2_sb[:], ls_sb[:], AF.Exp,
                                           scale=0.5)
                ins.then_inc(s_act, 1)
            else:
                tk.inc("act", ("p2_exp",))
            # zpart copy
            em.wait("pe", ("p2_zpart", 3))
            if em.real:
                ins = nc.scalar.copy(zpart_sb[:], z_ps[:])
                ins.then_inc(s_act, 1)
            else:
                tk.inc("act", ("p2_zpart_copy",))
            # dec
            for t in range(T):
                for l in range(3):
                    act_layer(em, "dec", t, l)
                em.wait("pe", ("proj", t))
                if em.real:
                    ins = nc.scalar.copy(pre_buf[:, t * V:(t + 1) * V],
                                         preP[t & 1][:, 0:V])
                    ins.then_inc(s_act, 1)
                else:
                    tk.inc("act", ("precopy", t))
            # final
            em.wait("dve", ("f_picked",))
            if em.real:
                nc.scalar.activation(pre_buf[:], pre_buf[:],
                                     AF.Exp).then_inc(s_act, 1)
            else:
                tk.inc("act", ("f_exp",))
            em.wait("dve", ("f_se",))
            if em.real:
                nc.scalar.activation(se_sb[:], se_sb[:], AF.Ln).then_inc(s_act, 1)
            else:
                tk.inc("act", ("f_ln",))
            if em.real:
                nc.scalar.activation(m2_sb[:], mean_sb[:], AF.Square)
                nc.scalar.activation(els_sb[:], ls_sb[:],
                                     AF.Exp).then_inc(s_act, 1)
            else:
                tk.inc("act", ("f_m2els",))
            # out copy
            em.wait("pe", ("f_red",))
            if em.real:
                nc.scalar.copy(out_sb[:], preP[0][0:1, 300:304]).then_inc(
                    s_act, 1)
            else:
                tk.inc("act", ("f_out",))

        # ---- DVE program ---------------------------------------------------
        def dve_layer(em, ph, t, l, xcount):
            g = gg_enc(t, l) if ph == "enc" else gg_dec(t, l)
            par = g & 1
            if l == 0:
                em.wait_val(f"xc{xcount % XC_BUFS}",
                            16 * (xcount // XC_BUFS + 1))
                for n in range(NBANK):
                    em.wait("pe", ("bank", ph, t, 0, n))
                    sl = slice(n * 512, (n + 1) * 512)
                    if em.real:
                        ins = nc.vector.tensor_tensor(
                            z_ps[:, sl], z_ps[:, sl],
                            xc_sb[(xcount) % XC_BUFS][:, sl], OP.add)
                        if ph == "dec":
                            ins = nc.vector.tensor_tensor(
                                z_ps[:, sl], z_ps[:, sl], zpart_sb[:, sl],
                                OP.add)
                        ins.then_inc(s_dve, 1)
                    else:
                        tk.inc("dve", ("xcadd", ph, t, n))
            em.wait("act", ("ifg", ph, t, l))
            if em.real:
                nc.vector.tensor_tensor(fc_sb[:], sig_if[par][:, U:2 * U],
                                        cst[l][:], OP.mult)
                ins = nc.vector.tensor_tensor(ig_sb[:], sig_if[par][:, 0:U],
                                              tanh_g[par][:], OP.mult)
                ins.then_inc(s_dve, 1)
                ins = nc.vector.tensor_tensor(cst[l][:], fc_sb[:], ig_sb[:],
                                              OP.add)
                ins.then_inc(s_dve, 1)
            else:
                tk.inc("dve", ("c2g", g))
                tk.inc("dve", ("c2", ph, t, l))
            em.wait("act", ("oc", ph, t, l))
            if g >= 2:
                em.wait("pe", ("tr", g - 2))
            if em.real:
                ins = nc.vector.tensor_tensor(h2_sb[par][:], sig_o[par][:],
                                              tanhc[par][:], OP.mult)
                ins.then_inc(s_dve, 2)
            else:
                tk.inc("dve", ("h2g", g))
                tk.inc("dve", ("h2", ph, t, l))
            em.wait("pe", ("tr", g))
            for j in range(KT):
                if em.real:
                    ins = nc.vector.tensor_copy(hT[l][:, j * BL:(j + 1) * BL],
                                                trP[par][:, j * BL:(j + 1) * BL])
                    if j == KT - 1:
                        ins.then_inc(s_dve, 1)
                elif j == KT - 1:
                    tk.inc("dve", copies_tag(g))

        def dve_body(e, real):
            em = Em(real, e)
            # init: zero states + partials
            if em.real:
                for l in range(3):
                    nc.vector.memset(hT[l][:], 0)
                    nc.vector.memset(cst[l][:], 0)
                nc.vector.memset(partial_sb[:], 0).then_inc(s_dve, 1)
            else:
                tk.inc("dve", ("init",))
            for t in range(T):
                for l in range(3):
                    dve_layer(em, "enc", t, l, t)
            # phase 2
            em.wait("pe", ("tr", GG_CT))
            for j in range(KT):
                if em.real:
                    ins = nc.vector.tensor_copy(
                        cT_sb[:, j * BL:(j + 1) * BL],
                        trP[GG_CT & 1][:, j * BL:(j + 1) * BL])
                    if j == KT - 1:
                        ins.then_inc(s_dve, 1)
                elif j == KT - 1:
                    tk.inc("dve", ("p2_cT",))
            em.wait("act", ("p2_exp",))
            if em.real:
                nc.vector.tensor_tensor(zr_sb[:], els2_sb[:], eps_sb[:], OP.mult)
                ins = nc.vector.tensor_tensor(zr_sb[:], zr_sb[:], mean_sb[:],
                                              OP.add)
                ins.then_inc(s_dve, 1)
            else:
                tk.inc("dve", ("p2_zr",))
            em.wait("pe", ("tr", GG_ZT))
            if em.real:
                nc.vector.tensor_copy(zT_sb[:, 0:BL], trP[GG_ZT & 1][:, 0:BL])
                ins = nc.vector.tensor_copy(
                    zT_sb[0:LAT - 128, BL:2 * BL],
                    trP[GG_ZT & 1][0:LAT - 128, BL:2 * BL])
                ins.then_inc(s_dve, 1)
            else:
                tk.inc("dve", ("p2_zT",))
            # re-zero states for decoder (hT last read by enc L2 final MMs)
            em.wait("pe", ("bank", "enc", T - 1, 2, 3))
            if em.real:
                for l in range(3):
                    nc.vector.memset(hT[l][:], 0)
                nc.vector.memset(cst[0][:], 0)
                nc.vector.memset(cst[1][:], 0)
                nc.vector.memset(cst[2][:], 0).then_inc(s_dve, 1)
            else:
                tk.inc("dve", ("init2",))
            for t in range(T):
                for l in range(3):
                    dve_layer(em, "dec", t, l, T + t)
            # final
            em.wait("act", ("precopy", T - 1))
            if em.real:
                nc.vector.tensor_tensor(ohm_sb[:], pre_buf[:], ohm_sb[:],
                                        OP.mult)
                ins = nc.vector.tensor_reduce(
                    out=partial_sb[:, 1:2], in_=ohm_sb[:],
                    axis=mybir.AxisListType.X, op=OP.add)
                ins.then_inc(s_dve, 1)
            else:
                tk.inc("dve", ("f_picked",))
            em.wait("act", ("f_exp",))
            if em.real:
                ins = nc.vector.tensor_reduce(
                    out=se_sb[:],
                    in_=pre_buf[:].rearrange("p (t v) -> p t v", v=V),
                    axis=mybir.AxisListType.X, op=OP.add)
                ins.then_inc(s_dve, 1)
            else:
                tk.inc("dve", ("f_se",))
            em.wait("act", ("f_ln",))
            if em.real:
                nc.vector.tensor_tensor(se_sb[:], se_sb[:], maskf_sb[:],
                                        OP.mult)
                ins = nc.vector.tensor_reduce(
                    out=partial_sb[:, 0:1], in_=se_sb[:],
                    axis=mybir.AxisListType.X, op=OP.add)
                ins.then_inc(s_dve, 1)
            else:
                tk.inc("dve", ("f_mlse",))
            em.wait("act", ("f_m2els",))
            if em.real:
                # kl_el = 1 + ls - m^2 - e^ls  (tiny per-element; the +1 must
                # be folded in before the fp32 sum or it swamps the signal)
                nc.vector.scalar_tensor_tensor(
                    out=els2_sb[:], in0=m2_sb[:], scalar=-1.0, in1=ls_sb[:],
                    op0=OP.mult, op1=OP.add)
                nc.vector.tensor_tensor(els_sb[:], els2_sb[:], els_sb[:],
                                        OP.subtract)
                nc.vector.tensor_scalar_add(els_sb[:], els_sb[:], 1.0)
                ins = nc.vector.tensor_reduce(
                    out=partial_sb[:, 2:3], in_=els_sb[:],
                    axis=mybir.AxisListType.X, op=OP.add)
                ins.then_inc(s_dve, 1)
            else:
                tk.inc("dve", ("f_kl",))

        # ---- SYNC (DMA) program -------------------------------------------
        def sync_body(e, real):
            em = Em(real, e)

            def dma(dst, src, sem, tag):
                if em.real:
                    e.dma_start(out=dst, in_=src).then_inc(SEMS[sem], 16)
                else:
                    tk.inc(sem, tag, 16)

            # small consts + tables
            dma(iden_sb[:], iden_d[:], "w", "w_iden")
            dma(ones32_sb[:], ones32_d[:], "w", "w_ones32")
            dma(ohm_sb[:], ohm_d[:], "w", "w_ohm")
            dma(maskf_sb[:], maskf_d[:], "w", "w_maskf")
            dma(eps_sb[:], eps_d[:], "w", "w_eps")
            dma(wm_sb[:], wm_d[:].rearrange("p a b -> p (a b)"), "w", "w_wm")
            dma(ws_sb[:], ws_d[:].rearrange("p a b -> p (a b)"), "w", "w_ws")
            dma(wo_sb[:], wo_d[:].rearrange("p a b -> p (a b)"), "w", "w_wo")
            dma(dk0z_sb[:], dk0z_d[:].rearrange("p a b -> p (a b)"), "w",
                "w_dk0z")
            for nm in bias_sb:
                dma(bias_sb[nm][:], bias_d[nm][:], "w", f"w_{nm}")
            dma(rk0_sb[:], rk0e_d[:].rearrange("p a b -> p (a b)"), "w",
                "w_rk0e")
            dma(w1_sb[:], w1e_d[:].rearrange("p a b -> p (a b)"), "w", "w_w1e")
            dma(w2_sb[:], w2e_d[:].rearrange("p a b -> p (a b)"), "w", "w_w2e")
            # xc stream (enc then dec share the ring)
            for t in range(2 * T):
                ph = "enc" if t < T else "dec"
                tt = t if t < T else t - T
                if t >= XC_BUFS:
                    pt = t - XC_BUFS
                    em.wait("dve", ("xcadd", "enc" if pt < T else "dec",
                                    pt if pt < T else pt - T, 3))
                src = xc_enc_d if ph == "enc" else xc_dec_d
                dma(xc_sb[t % XC_BUFS][:], src[tt], f"xc{t % XC_BUFS}",
                    ("xc", t))
                # after the last encoder xc load, queue the dec weight swap
                if t == T - 1:
                    em.wait("pe", ("bank", "enc", T - 1, 2, 3))
                    dma(rk0_sb[:], rk0d_d[:].rearrange("p a b -> p (a b)"),
                        "w", "w_rk0d")
                    dma(w1_sb[:], w1d_d[:].rearrange("p a b -> p (a b)"),
                        "w", "w_w1d")
                    dma(w2_sb[:], w2d_d[:].rearrange("p a b -> p (a b)"),
                        "w", "w_w2d")
            # final out
            em.wait("act", ("f_out",))
            if DEBUG_DUMP:
                em.wait("dve", ("f_kl",))
                dma(dbg_d[:, 0:T], se_sb[:], "out", "dbg1")
                dma(dbg_d[:, T:2 * T], maskf_sb[:], "out", "dbg2")
            dma(out_d[:], out_sb[:], "out", "out")
            if em.real:
                e.wait_ge(s_out, 48 if DEBUG_DUMP else 16)

        # "w_dec" composite tag: value after the 3 dec weight DMAs
        # (recorded during pass 1 as w_w2d)
        # pass 1 (record)
        pe_body(None, False)
        act_body(None, False)
        dve_body(None, False)
        sync_body(None, False)
        # DMA completions are not ordered across queues, so every weight
        # threshold waits for ALL dmas of its phase (count-based, order-free)
        tk.vals[("w", ("w_dec",))] = tk.vals[("w", "w_w2d")]
        tk.vals[("w", ("w_rk0e",))] = tk.vals[("w", "w_w2e")]
        tk.vals[("w", ("w_w1e",))] = tk.vals[("w", "w_w2e")]
        tk.vals[("w", ("w_w2e",))] = tk.vals[("w", "w_w2e")]
        # phase-2 transpose groups alias into the generic copies-tag chain
        tk.vals[("dve", ("copies", GG_CT))] = tk.vals[("dve", ("p2_cT",))]
        tk.vals[("dve", ("copies", GG_ZT))] = tk.vals[("dve", ("p2_zT",))]

        # pass 2 (emit)
        @blk.tensor
        def _(e):
            pe_body(e, True)

        @blk.scalar
        def _(e):
            act_body(e, True)

        @blk.vector
        def _(e):
            dve_body(e, True)

        @blk.sync
        def _(e):
            sync_body(e, True)

    return nc


def _prep(inputs):
    """Host-side preprocessing -> per-core input maps + flags."""
    X = np.asarray(inputs["X"]).astype(np.int64)
    Y = np.asarray(inputs["Y"]).astype(np.int64)
    C = np.asarray(inputs["C"]).astype(np.float32)
    L = np.asarray(inputs["L"]).astype(np.int64)
    eps = np.asarray(inputs["eps"]).astype(np.float32)
    f = lambda n: np.asarray(inputs[n]).astype(np.float32)
    emb_enc, emb_dec = f("emb_enc"), f("emb_dec")
    enc_k0, enc_b0 = f("enc_k0"), f("enc_b0")
    dec_k0, dec_b0 = f("dec_k0"), f("dec_b0")

    def ktiles(w, kt):  # [kt*128, n] -> [128, kt, n]
        n = w.shape[1]
        return np.ascontiguousarray(
            w.reshape(kt, 128, n).transpose(1, 0, 2)).astype(bf16)

    def ktiles_pad(w, kt):  # pad rows to kt*128 first
        r = kt * 128 - w.shape[0]
        if r:
            w = np.vstack([w, np.zeros((r, w.shape[1]), w.dtype)])
        return ktiles(w, kt)

    table_enc = emb_enc @ enc_k0[:LAT]            # [42, 2048]
    table_dec = emb_dec @ dec_k0[LAT:2 * LAT]

    flags = {
        "b1e": bool(np.any(inputs["enc_b1"])), "b2e": bool(np.any(inputs["enc_b2"])),
        "b1d": bool(np.any(inputs["dec_b1"])), "b2d": bool(np.any(inputs["dec_b2"])),
        "bm": bool(np.any(inputs["bm"])), "bs": bool(np.any(inputs["bs"])),
        "bo": bool(np.any(inputs["bo"])),
    }

    shared = {
        "rk0e": ktiles(f("enc_rk0"), KT),
        "w1e": ktiles(np.vstack([f("enc_k1"), f("enc_rk1")]), 2 * KT),
        "w2e": ktiles(np.vstack([f("enc_k2"), f("enc_rk2")]), 2 * KT),
        "rk0d": ktiles(f("dec_rk0"), KT),
        "w1d": ktiles(np.vstack([f("dec_k1"), f("dec_rk1")]), 2 * KT),
        "w2d": ktiles(np.vstack([f("dec_k2"), f("dec_rk2")]), 2 * KT),
        "dk0z": ktiles_pad(dec_k0[:LAT], 2),
        "wm": ktiles(f("Wm"), KT),
        "ws": ktiles(f("Ws"), KT),
        "wo": ktiles(f("Wo"), KT),
        "iden": np.eye(BL, dtype=np.float32),
        "ones32": np.ones((BL, 1), np.float32),
    }
    if flags["b1e"]:
        shared["b1e"] = np.asarray(inputs["enc_b1"]).reshape(1, -1).astype(bf16)
    if flags["b2e"]:
        shared["b2e"] = np.asarray(inputs["enc_b2"]).reshape(1, -1).astype(bf16)
    if flags["b1d"]:
        shared["b1d"] = np.asarray(inputs["dec_b1"]).reshape(1, -1).astype(bf16)
    if flags["b2d"]:
        shared["b2d"] = np.asarray(inputs["dec_b2"]).reshape(1, -1).astype(bf16)
    if flags["bm"]:
        shared["bm"] = f("bm").reshape(1, -1).astype(bf16)
    if flags["bs"]:
        shared["bs"] = f("bs").reshape(1, -1).astype(bf16)
    if flags["bo"]:
        shared["bo"] = f("bo").reshape(1, -1).astype(bf16)
    if any(flags.values()):
        shared["ones1"] = np.ones((1, BL), bf16)

    in_maps = []
    tt = np.arange(T)
    vv = np.arange(V)
    for c in range(NCORES):
        sl = slice(c * BL, (c + 1) * BL)
        Xl, Yl, Cl, Ll, epsl = X[sl], Y[sl], C[sl], L[sl], eps[sl]
        cpart_e = Cl @ enc_k0[LAT:] + enc_b0      # [32, 2048]
        cpart_d = Cl @ dec_k0[2 * LAT:] + dec_b0
        xce = table_enc[Xl] + cpart_e[:, None, :]     # [32, 128, 2048]
        xcd = table_dec[Xl] + cpart_d[:, None, :]
        xce = np.ascontiguousarray(xce.transpose(1, 0, 2)).astype(bf16)
        xcd = np.ascontiguousarray(xcd.transpose(1, 0, 2)).astype(bf16)
        ohm = ((Yl[:, :, None] == vv) &
               (tt[None, :, None] < Ll[:, None, None]))
        ohm = ohm.reshape(BL, T * V).astype(bf16)
        maskf = (tt[None, :] < Ll[:, None]).astype(np.float32)
        m = dict(shared)
        m.update(xc_enc=xce, xc_dec=xcd, ohm=ohm, maskf=maskf,
                 eps=np.ascontiguousarray(epsl))
        in_maps.append(m)
    return in_maps, flags


_BUILD_CACHE = {}


def kernel(**inputs):
    in_maps, flags = _prep(inputs)
    key = tuple(sorted(flags.items()))
    if key not in _BUILD_CACHE:
        _BUILD_CACHE[key] = _build(flags)
    nc = _BUILD_CACHE[key]
    res = run_bass_kernel_spmd(nc, in_maps, list(range(NCORES)))
    mlse = picked = kl = 0.0
    global _LAST_PARTIALS, _LAST_DBG
    _LAST_PARTIALS = []
    _LAST_DBG = [np.asarray(res.results[c].get("dbg")) for c in range(NCORES)] if DEBUG_DUMP else None
    for c in range(NCORES):
        o = np.asarray(res.results[c]["out"], np.float64).reshape(-1)
        _LAST_PARTIALS.append(o.copy())
        mlse += o[0]
        picked += o[1]
        kl += o[2]
    recon = (mlse - picked) / (B * T)
    latent = -0.5 * (kl / (B * LAT))
    loss = recon + latent
    return (np.float32(loss), np.float32(recon), np.float32(latent))

